# revision 9
# baseline (speedup 1.0000x reference)
"""Trainium2 Bass kernel for 16-head self-attention (B=2, S=2048, D=1024).

Sharding: 8 cores = 2 batches x 4 head-groups (4 heads each).  Wq/Wk/Wv are
column-split, Wo row-split (tensor parallel over heads) + data parallel over
batch.  Each core computes a partial [S, D] output (bf16); host sums the 4
partials per batch (the TP reduce) and stacks the 2 batches.

Host-side prep (layout only, no model FLOPs): q/k/v are transposed to
d-major [D, S] and cast to bf16; the mask is transposed to [keys, queries]
bf16; the four weight matrices are packed bf16 into one [128, 8192] operand.
This removes all PE transposes from the device program and cuts DMA traffic.

Per-core device pipeline (PE matmuls contract along SBUF partitions):
  1. Projections from resident xT tiles: vb = x @ Wv.T s-major [S, 256];
     kT/qT = (x @ W.T).T d-major [256, S]; all bf16.
  2. Attention per (q-chunk j, key-chunk kc): scoresT[k,q] = kT-slice.T @ qT
     (two heads packed in PE rows via tile_position), exp via ACT (1/8 scale
     folded) -> bf16, multiply by resident transposed mask (DVE), then
     ctx[dk,q] += v.T @ attnT (col-packed head pairs) and denom[q] += ones.T
     @ attnT (M=1, 4-way col-packed), accumulated in PSUM over kc.  Softmax
     max-subtraction is skipped: scores ~ N(0,1) so fp32 exp is safe, and
     masked entries are exactly zeroed by the multiply.
  3. Normalize per j: reciprocal of denom rows -> partition-broadcast via K=1
     outer-product matmul -> multiply into ctxT on PSUM eviction.
  4. Output projection per j (overlaps attention of j+1): out-rows = ctxT.T @
     woT accumulated over 2 dk-chunks, evict bf16, DMA out.
"""

import sys
from contextlib import ExitStack

import numpy as np

sys.path.insert(0, "/opt/trn_rl_repo")

import concourse.bacc as bacc
import concourse.bass as bass
import concourse.mybir as mybir
import concourse.tile as tile
from concourse.bass import ds, ts

B, S, D, H = 2, 2048, 1024, 16
DK = D // H  # 64
NCORES = 8
GH = H // (NCORES // B)  # 4 heads per core
GD = GH * DK  # 256 projected dims per core

F32 = mybir.dt.float32
BF16 = mybir.dt.bfloat16

P = 128
NQ = 512  # q free-dim chunk in the attention loop
WFREE = 3 * (D // P) * GD + (GD // P) * D  # 8192 packed weight cols


def build_nc(s=S, d=D, gh=GH, dk=DK, sim=False, phases=3):
    gd = gh * dk
    SC = s // P  # 128-row chunks (also key chunks)
    DC = d // P
    GDC = gd // P
    JC = s // NQ
    HPAIRS = gh // 2
    scale = float(1.0 / np.sqrt(dk))

    nc = bacc.Bacc("TRN2", target_bir_lowering=False, debug=sim)
    xT3 = nc.dram_tensor("xT3", [3, d, s], BF16, kind="ExternalInput")
    maskT = nc.dram_tensor("maskT", [s, s], BF16, kind="ExternalInput")
    wpk = nc.dram_tensor("wpk", [P, WFREE], BF16, kind="ExternalInput")
    out = nc.dram_tensor("out", [s, d], BF16, kind="ExternalOutput")
    dbg = {}
    if phases < 3:
        dbg["qT"] = nc.dram_tensor("dbg_qT", [P, GDC, s], BF16, kind="ExternalOutput")
        dbg["kT"] = nc.dram_tensor("dbg_kT", [P, GDC, s], BF16, kind="ExternalOutput")
        dbg["vb"] = nc.dram_tensor("dbg_vb", [P, SC, gd], BF16, kind="ExternalOutput")
    if phases == 2:
        dbg["ctxT"] = nc.dram_tensor("dbg_ctxT", [P, GDC, s], BF16, kind="ExternalOutput")

    # packed weight slices: wq/wk/wv at proj*DC*gd, row-chunk kc at kc*gd;
    # wo at 3*DC*gd, dk-chunk kc at kc*d.
    def w_qkv(wt, proj, kc):
        return wt[:, ds(proj * DC * gd + kc * gd, gd)]

    def w_o(wt, kc):
        return wt[:, ds(3 * DC * gd + kc * d, d)]

    with tile.TileContext(nc) as tc, ExitStack() as top:
        consts = top.enter_context(tc.tile_pool(name="consts", bufs=1))
        sb = top.enter_context(tc.tile_pool(name="sb", bufs=1))
        xpool = top.enter_context(tc.tile_pool(name="xpool", bufs=2))
        ctxp = top.enter_context(tc.tile_pool(name="ctxp", bufs=2))
        attnp = top.enter_context(tc.tile_pool(name="attnp", bufs=4))
        smalls = top.enter_context(tc.tile_pool(name="smalls", bufs=2))
        outp = top.enter_context(tc.tile_pool(name="outp", bufs=3))
        ps_acc = top.enter_context(tc.tile_pool(name="ps_acc", bufs=1, space="PSUM"))
        ps_sc = top.enter_context(tc.tile_pool(name="ps_sc", bufs=2, space="PSUM"))
        ps_gen = top.enter_context(tc.tile_pool(name="ps_gen", bufs=1, space="PSUM"))

        onesb = consts.tile([P, 1], BF16)
        nc.any.memset(onesb[:], 1.0)
        onesf = consts.tile([P, P], F32)
        nc.any.memset(onesf[:], 1.0)

        wt = sb.tile([P, WFREE], BF16, tag="wt")
        nc.sync.dma_start(wt[:], wpk[:])
        mT = sb.tile([P, SC, s], BF16, tag="mT")  # maskT resident [keys, q]
        nc.gpsimd.dma_start(mT[:], maskT.rearrange("(c p) q -> p c q", p=P))

        qT = sb.tile([P, GDC, s], BF16, tag="qT")  # [gd, s] d-major
        kT = sb.tile([P, GDC, s], BF16, tag="kT")
        vb = sb.tile([P, SC, gd], BF16, tag="vb")  # [s, gd] s-major

        # ======================= phase 1: projections =======================
        evict_pick = [0]

        def evict(dst, src):
            # PSUM evictions: only ACT and DVE can read PSUM.
            e = evict_pick[0] % 2
            evict_pick[0] += 1
            if e == 0:
                nc.vector.tensor_copy(dst, src)
            else:
                nc.scalar.copy(dst, src)

        def load_xt(proj):
            xt = xpool.tile([P, DC, s], BF16, tag="xt", name=f"xt{proj}")
            nc.sync.dma_start(xt[:], xT3[proj].rearrange("(c p) s -> p c s", p=P))
            return xt

        # v: s-major
        xt = load_xt(2)
        for sc in range(SC):
            pp = ps_sc.tile([P, NQ], F32, tag="sc", name=f"ppv_{sc}")
            for kc in range(DC):
                nc.tensor.matmul(
                    pp[:, :gd],
                    xt[:, kc, ts(sc, P)],
                    w_qkv(wt, 2, kc),
                    start=(kc == 0),
                    stop=(kc == DC - 1),
                )
            evict(vb[:, sc], pp[:, :gd])

        # k then q: d-major
        for proj, dst in ((1, kT), (0, qT)):
            xt = load_xt(proj)
            for j in range(JC):
                for mc in range(GDC):
                    pp = ps_sc.tile([P, NQ], F32, tag="sc", name=f"pp{proj}_{j}_{mc}")
                    for kc in range(DC):
                        nc.tensor.matmul(
                            pp[:],
                            w_qkv(wt, proj, kc)[:, ts(mc, P)],
                            xt[:, kc, ds(j * NQ, NQ)],
                            start=(kc == 0),
                            stop=(kc == DC - 1),
                        )
                    evict(dst[:, mc, ds(j * NQ, NQ)], pp[:])

        if phases < 3:
            nc.sync.dma_start(dbg["qT"][:], qT[:])
            nc.sync.dma_start(dbg["kT"][:], kT[:])
            nc.sync.dma_start(dbg["vb"][:], vb[:])
        if phases < 2:
            nc.finalize() if not sim else nc.compile()
            return nc

        # ================== phase 2+3: attention + out-proj ==================
        for j in range(JC):
            ctx_ps = [
                ps_acc.tile([P, NQ], F32, tag=f"ctx{hp}", name=f"ctx_ps{hp}_{j}")
                for hp in range(HPAIRS)
            ]
            den_ps = ps_acc.tile([P, NQ], F32, tag="den")

            for kc in range(SC):
                ats = []
                for hp in range(HPAIRS):
                    heads = (2 * hp, 2 * hp + 1)
                    sc_ps = ps_sc.tile([P, 2, NQ], F32, tag="sc", name=f"sc{hp}_{j}_{kc}")
                    for i, h in enumerate(heads):
                        mc, off = divmod(h * dk, P)
                        nc.tensor.matmul(
                            sc_ps[:, i],
                            kT[:, mc, ts(kc, P)][ds(off, dk), :],
                            qT[:, mc, ds(j * NQ, NQ)][ds(off, dk), :],
                            start=True,
                            stop=True,
                            tile_position=(off, 0),
                        )
                    at = attnp.tile([P, 2, NQ], BF16, tag="at", name=f"at{hp}_{j}_{kc}")
                    nc.scalar.activation(
                        at[:], sc_ps[:], mybir.ActivationFunctionType.Exp,
                        scale=scale,
                    )
                    for i in range(2):
                        nc.vector.tensor_tensor(
                            at[:, i], at[:, i], mT[:, kc, ds(j * NQ, NQ)],
                            op=mybir.AluOpType.mult,
                        )
                    ats.append(at)
                for h in range(gh):
                    hp, i = divmod(h, 2)
                    nc.tensor.matmul(
                        ctx_ps[hp][ds(i * dk, dk), :],
                        vb[:, kc, ds(h * dk, dk)],
                        ats[hp][:, i],
                        start=(kc == 0),
                        stop=(kc == SC - 1),
                        tile_position=(0, i * dk),
                        skip_group_check=True,
                    )
                for h in range(gh):
                    hp, i = divmod(h, 2)
                    nc.tensor.matmul(
                        den_ps[ds(32 * h, 1), :],
                        onesb[:, :],
                        ats[hp][:, i],
                        start=(kc == 0),
                        stop=(kc == SC - 1),
                        tile_position=(0, 32 * h),
                        skip_group_check=True,
                    )

            # normalize into ctxT for this q-chunk
            ctxT = ctxp.tile([P, GDC, NQ], BF16, tag="ctxT", name=f"ctxT_{j}")
            rec = smalls.tile([P, NQ], F32, tag="rec")
            for h in range(gh):
                nc.vector.reciprocal(rec[ds(32 * h, 1), :], den_ps[ds(32 * h, 1), :])
            for hp in range(HPAIRS):
                heads = (2 * hp, 2 * hp + 1)
                bc_ps = ps_acc.tile([P, NQ], F32, tag="den", name=f"bc_{j}_{hp}")
                for i, h in enumerate(heads):
                    nc.tensor.matmul(
                        bc_ps[ds(i * dk, dk), :],
                        onesf[ds(32 * h, 1), :dk],
                        rec[ds(32 * h, 1), :],
                        start=True,
                        stop=True,
                        tile_position=(32 * h, i * dk),
                        skip_group_check=True,
                    )
                bc_sb = smalls.tile([P, NQ], F32, tag="bcs")
                nc.scalar.copy(bc_sb[:], bc_ps[:])
                nc.vector.tensor_tensor(
                    ctxT[:, hp, :],
                    ctx_ps[hp][:],
                    bc_sb[:],
                    op=mybir.AluOpType.mult,
                )

            if phases == 2:
                nc.sync.dma_start(dbg["ctxT"][:, :, ds(j * NQ, NQ)], ctxT[:])
                continue

            # out-proj for this q-chunk: 4 row-chunks of 128
            for scl in range(NQ // P):
                ot = outp.tile([P, d], BF16, tag="ot", name=f"ot_{j}_{scl}")
                for nj in range(d // NQ):
                    po = ps_gen.tile([P, NQ], F32, tag="pp", name=f"po_{j}_{scl}_{nj}")
                    for kc in range(GDC):
                        nc.tensor.matmul(
                            po[:],
                            ctxT[:, kc, ts(scl, P)],
                            w_o(wt, kc)[:, ds(nj * NQ, NQ)],
                            start=(kc == 0),
                            stop=(kc == GDC - 1),
                        )
                    if nj % 2 == 0:
                        nc.scalar.copy(ot[:, ds(nj * NQ, NQ)], po[:])
                    else:
                        nc.vector.tensor_copy(ot[:, ds(nj * NQ, NQ)], po[:])
                nc.sync.dma_start(out[ts(j * (NQ // P) + scl, P), :], ot[:])

    if sim:
        nc.compile()
    else:
        nc.finalize()
    return nc


_NC_CACHE = {}


def get_nc(**kw):
    key = tuple(sorted(kw.items()))
    if key not in _NC_CACHE:
        _NC_CACHE[key] = build_nc(**kw)
    return _NC_CACHE[key]


def _bf16(a):
    import ml_dtypes

    return np.ascontiguousarray(a.astype(ml_dtypes.bfloat16))


def shard_inputs(q, k, v, mask, Wq, Wk, Wv, Wo):
    q = np.asarray(q, dtype=np.float32)
    k = np.asarray(k, dtype=np.float32)
    v = np.asarray(v, dtype=np.float32)
    mask = np.asarray(mask, dtype=np.int32)
    Wq, Wk, Wv, Wo = (np.asarray(w, dtype=np.float32) for w in (Wq, Wk, Wv, Wo))

    # per-batch shared tensors
    xT3 = [_bf16(np.stack([q[b].T, k[b].T, v[b].T])) for b in range(B)]
    mTb = [_bf16(mask[b, 0].T.astype(np.float32)) for b in range(B)]

    def pack_w(rows):
        parts = []
        for W in (Wq, Wk, Wv):
            wT = W[rows, :].T  # [D, GD]
            parts.append(wT.reshape(D // P, P, GD).transpose(1, 0, 2).reshape(P, -1))
        woT = Wo[:, rows].T  # [GD, D]
        parts.append(woT.reshape(GD // P, P, D).transpose(1, 0, 2).reshape(P, -1))
        return _bf16(np.concatenate(parts, axis=1))

    in_maps = []
    for c in range(NCORES):
        b, g = divmod(c, NCORES // B)
        rows = slice(g * GD, (g + 1) * GD)
        in_maps.append({"xT3": xT3[b], "maskT": mTb[b], "wpk": pack_w(rows)})
    return in_maps


def kernel(q, k, v, mask, Wq, Wk, Wv, Wo):
    from concourse.bass_utils import run_bass_kernel_spmd

    nc = get_nc()
    in_maps = shard_inputs(q, k, v, mask, Wq, Wk, Wv, Wo)
    res = run_bass_kernel_spmd(nc, in_maps, list(range(NCORES))).results
    out = np.zeros((B, S, D), dtype=np.float32)
    for c in range(NCORES):
        out[c // (NCORES // B)] += np.asarray(res[c]["out"]).astype(np.float32)
    return out


if __name__ == "__main__":
    nc = build_nc()
    print("built ok")


# revision 29
# speedup vs baseline: 1.2606x; 1.2606x over previous
"""Trainium2 Bass kernel for 16-head self-attention (B=2, S=2048, D=1024).

Sharding: 8 cores = 2 batches x 4 head-groups (4 heads each).  Wq/Wk/Wv are
column-split, Wo row-split (tensor parallel over heads) + data parallel over
batch.  Each core computes a partial [S, D] output (bf16); host sums the 4
partials per batch (the TP reduce) and stacks the 2 batches.

Host-side prep (layout only, no model FLOPs): q/k/v are transposed to
d-major [D, S] and cast to bf16; the mask is transposed to [keys, queries]
bf16; the four weight matrices are packed bf16 into one [128, 8192] operand.
This removes all PE transposes from the device program and cuts DMA traffic.

Per-core device pipeline (PE matmuls contract along SBUF partitions):
  1. Projections from resident xT tiles: vb = x @ Wv.T s-major [S, 256];
     kT = (x @ Wk.T).T d-major [256, S]; qT likewise but projected one
     q-chunk at a time inside the attention loop so exp starts earlier and
     q-proj PE work hides under the ACT-bound attention phase; all bf16.
  2. Attention per (q-chunk j, key-chunk kc): scoresT[k,q] = kT-slice.T @ qT
     (two heads packed in PE rows via tile_position), exp via ACT (1/8 scale
     folded) -> bf16, multiply by resident transposed mask (DVE), then
     ctx[dk,q] += v.T @ attnT (col-packed head pairs) and denom[q] += ones.T
     @ attnT (M=1, 4-way col-packed), accumulated in PSUM over kc.  Softmax
     max-subtraction is skipped: scores ~ N(0,1) so fp32 exp is safe, and
     masked entries are exactly zeroed by the multiply.
  3. Normalize per j: reciprocal of denom rows -> partition-broadcast via K=1
     outer-product matmul -> multiply into ctxT on PSUM eviction.
  4. Output projection per j (overlaps attention of j+1): out-rows = ctxT.T @
     woT accumulated over 2 dk-chunks, evict bf16, DMA out.
"""

import sys
from contextlib import ExitStack

import numpy as np

sys.path.insert(0, "/opt/trn_rl_repo")

import concourse.bacc as bacc
import concourse.bass as bass
import concourse.mybir as mybir
import concourse.tile as tile
from concourse.bass import ds, ts

B, S, D, H = 2, 2048, 1024, 16
DK = D // H  # 64
NCORES = 8
GH = H // (NCORES // B)  # 4 heads per core
GD = GH * DK  # 256 projected dims per core

F32 = mybir.dt.float32
BF16 = mybir.dt.bfloat16

P = 128
NQ = 512  # q free-dim chunk in the attention loop
WFREE = 3 * (D // P) * GD + (GD // P) * D  # 8192 packed weight cols


def build_nc(s=S, d=D, gh=GH, dk=DK, sim=False, phases=3, out_dt="bf16"):
    gd = gh * dk
    SC = s // P  # 128-row chunks (also key chunks)
    DC = d // P
    GDC = gd // P
    JC = s // NQ
    HPAIRS = gh // 2
    scale = float(1.0 / np.sqrt(dk))

    nc = bacc.Bacc("TRN2", target_bir_lowering=False, debug=sim)
    xT3 = nc.dram_tensor("xT3", [3, d, s], BF16, kind="ExternalInput")
    maskT = nc.dram_tensor("maskT", [s, s], BF16, kind="ExternalInput")
    wpk = nc.dram_tensor("wpk", [P, WFREE], BF16, kind="ExternalInput")
    ODT = BF16 if out_dt == "bf16" else F32
    out = nc.dram_tensor("out", [s, d], ODT, kind="ExternalOutput")
    dbg = {}
    if phases < 3:
        dbg["qT"] = nc.dram_tensor("dbg_qT", [P, GDC, s], BF16, kind="ExternalOutput")
        dbg["kT"] = nc.dram_tensor("dbg_kT", [P, GDC, s], BF16, kind="ExternalOutput")
        dbg["vb"] = nc.dram_tensor("dbg_vb", [P, SC, gd], BF16, kind="ExternalOutput")
    if phases == 2:
        dbg["ctxT"] = nc.dram_tensor("dbg_ctxT", [P, GDC, s], BF16, kind="ExternalOutput")

    # packed weight slices: wq/wk/wv at proj*DC*gd, row-chunk kc at kc*gd;
    # wo at 3*DC*gd, dk-chunk kc at kc*d.
    def w_qkv(wt, proj, kc):
        return wt[:, ds(proj * DC * gd + kc * gd, gd)]

    def w_o(wt, kc):
        return wt[:, ds(3 * DC * gd + kc * d, d)]

    with tile.TileContext(nc) as tc, ExitStack() as top:
        consts = top.enter_context(tc.tile_pool(name="consts", bufs=1))
        sb = top.enter_context(tc.tile_pool(name="sb", bufs=1))
        xpool = top.enter_context(tc.tile_pool(name="xpool", bufs=2))
        ctxp = top.enter_context(tc.tile_pool(name="ctxp", bufs=2))
        attnp = top.enter_context(tc.tile_pool(name="attnp", bufs=6))
        smalls = top.enter_context(tc.tile_pool(name="smalls", bufs=3))
        outp = top.enter_context(tc.tile_pool(name="outp", bufs=4))
        ps_acc = top.enter_context(tc.tile_pool(name="ps_acc", bufs=1, space="PSUM"))
        ps_sc = top.enter_context(tc.tile_pool(name="ps_sc", bufs=2, space="PSUM"))
        ps_gen = top.enter_context(tc.tile_pool(name="ps_gen", bufs=1, space="PSUM"))

        onesb = consts.tile([P, 1], BF16)
        nc.any.memset(onesb[:], 1.0)
        onesf = consts.tile([P, P], F32)
        nc.any.memset(onesf[:], 1.0)

        wt = sb.tile([P, WFREE], BF16, tag="wt")
        mT = sb.tile([P, SC, s], BF16, tag="mT")  # maskT resident [keys, q]
        nc.gpsimd.dma_start(mT[:], maskT.rearrange("(c p) q -> p c q", p=P))

        qT = sb.tile([P, GDC, s], BF16, tag="qT")  # [gd, s] d-major
        kT = sb.tile([P, GDC, s], BF16, tag="kT")
        vb = sb.tile([P, SC, gd], BF16, tag="vb")  # [s, gd] s-major

        # phase-0 DMA schedule: v-proj needs wv + xt2 chunk0 first.
        nwq = DC * gd
        xt2 = xpool.tile([P, DC, s], BF16, tag="xt", name="xt2")
        xt2_src = xT3[2].rearrange("(c p) s -> p c s", p=P)
        for sl in (ds(0, s // 8), ds(s // 8, s // 8), ds(s // 4, s // 4)):
            nc.sync.dma_start(xt2[:, :, sl], xt2_src[:, :, sl])
        nc.scalar.dma_start(wt[:, ds(2 * nwq, nwq)], wpk[:, ds(2 * nwq, nwq)])  # wv
        nc.scalar.dma_start(wt[:, ds(1 * nwq, nwq)], wpk[:, ds(1 * nwq, nwq)])  # wk
        for q4 in range(2, 4):
            sl = ds(q4 * (s // 4), s // 4)
            nc.sync.dma_start(xt2[:, :, sl], xt2_src[:, :, sl])
        nc.scalar.dma_start(wt[:, ds(0, nwq)], wpk[:, ds(0, nwq)])  # wq
        nc.scalar.dma_start(  # wo
            wt[:, ds(3 * nwq, GDC * d)], wpk[:, ds(3 * nwq, GDC * d)]
        )

        # ======================= phase 1: projections =======================
        evict_pick = [0]

        def evict(dst, src):
            # PSUM evictions: only ACT and DVE can read PSUM.
            e = evict_pick[0] % 2
            evict_pick[0] += 1
            if e == 0:
                nc.vector.tensor_copy(dst, src)
            else:
                nc.scalar.copy(dst, src)

        def load_xt(proj):
            xt = xpool.tile([P, DC, s], BF16, tag="xt", name=f"xt{proj}")
            src = xT3[proj].rearrange("(c p) s -> p c s", p=P)
            for q4 in range(4):
                sl = ds(q4 * (s // 4), s // 4)
                eng = nc.sync if q4 % 2 == 0 else nc.scalar
                eng.dma_start(xt[:, :, sl], src[:, :, sl])
            return xt

        # v: s-major
        xt = xt2
        for sc in range(SC):
            pp = ps_sc.tile([P, NQ], F32, tag="sc", name=f"ppv_{sc}")
            for kc in range(DC):
                nc.tensor.matmul(
                    pp[:, :gd],
                    xt[:, kc, ts(sc, P)],
                    w_qkv(wt, 2, kc),
                    start=(kc == 0),
                    stop=(kc == DC - 1),
                )
            evict(vb[:, sc], pp[:, :gd])

        # k: d-major
        xt = load_xt(1)
        for j in range(JC):
            for mc in range(GDC):
                pp = ps_sc.tile([P, NQ], F32, tag="sc", name=f"ppk_{j}_{mc}")
                for kc in range(DC):
                    nc.tensor.matmul(
                        pp[:],
                        w_qkv(wt, 1, kc)[:, ts(mc, P)],
                        xt[:, kc, ds(j * NQ, NQ)],
                        start=(kc == 0),
                        stop=(kc == DC - 1),
                    )
                evict(kT[:, mc, ds(j * NQ, NQ)], pp[:])

        # q: projected per q-chunk inside the main loop (overlaps attention)
        xt0 = load_xt(0)

        def proj_q(j):
            for mc in range(GDC):
                pp = ps_sc.tile([P, NQ], F32, tag="sc", name=f"ppq_{j}_{mc}")
                for kc in range(DC):
                    nc.tensor.matmul(
                        pp[:],
                        w_qkv(wt, 0, kc)[:, ts(mc, P)],
                        xt0[:, kc, ds(j * NQ, NQ)],
                        start=(kc == 0),
                        stop=(kc == DC - 1),
                    )
                nc.vector.tensor_copy(qT[:, mc, ds(j * NQ, NQ)], pp[:])

        if phases < 2:
            for j in range(JC):
                proj_q(j)
            nc.sync.dma_start(dbg["qT"][:], qT[:])
            nc.sync.dma_start(dbg["kT"][:], kT[:])
            nc.sync.dma_start(dbg["vb"][:], vb[:])
            nc.finalize() if not sim else nc.compile()
            return nc

        # ================== phase 2+3: attention + out-proj ==================
        for j in range(JC):
            proj_q(j)
            ctx_ps = [
                ps_acc.tile([P, NQ], F32, tag=f"ctx{hp}", name=f"ctx_ps{hp}_{j}")
                for hp in range(HPAIRS)
            ]
            den_ps = ps_acc.tile([P, NQ], F32, tag="den")

            for kc in range(SC):
                ats = []
                for hp in range(HPAIRS):
                    heads = (2 * hp, 2 * hp + 1)
                    sc_ps = ps_sc.tile([P, 2, NQ], F32, tag="sc", name=f"sc{hp}_{j}_{kc}")
                    for i, h in enumerate(heads):
                        mc, off = divmod(h * dk, P)
                        nc.tensor.matmul(
                            sc_ps[:, i],
                            kT[:, mc, ts(kc, P)][ds(off, dk), :],
                            qT[:, mc, ds(j * NQ, NQ)][ds(off, dk), :],
                            start=True,
                            stop=True,
                            tile_position=(off, 0),
                        )
                    at = attnp.tile([P, 2, NQ], BF16, tag="at", name=f"at{hp}_{j}_{kc}")
                    nc.scalar.activation(
                        at[:], sc_ps[:], mybir.ActivationFunctionType.Exp,
                        scale=scale,
                    )
                    for i in range(2):
                        nc.vector.tensor_tensor(
                            at[:, i], at[:, i], mT[:, kc, ds(j * NQ, NQ)],
                            op=mybir.AluOpType.mult,
                        )
                    ats.append(at)
                for h in range(gh):
                    hp, i = divmod(h, 2)
                    nc.tensor.matmul(
                        ctx_ps[hp][ds(i * dk, dk), :],
                        vb[:, kc, ds(h * dk, dk)],
                        ats[hp][:, i],
                        start=(kc == 0),
                        stop=(kc == SC - 1),
                        tile_position=(0, i * dk),
                        skip_group_check=True,
                    )
                for h in range(gh):
                    hp, i = divmod(h, 2)
                    nc.tensor.matmul(
                        den_ps[ds(32 * h, 1), :],
                        onesb[:, :],
                        ats[hp][:, i],
                        start=(kc == 0),
                        stop=(kc == SC - 1),
                        tile_position=(0, 32 * h),
                        skip_group_check=True,
                    )

            # normalize into ctxT for this q-chunk
            ctxT = ctxp.tile([P, GDC, NQ], BF16, tag="ctxT", name=f"ctxT_{j}")
            rec = smalls.tile([P, NQ], F32, tag="rec")
            for h in range(gh):
                nc.vector.reciprocal(rec[ds(32 * h, 1), :], den_ps[ds(32 * h, 1), :])
            for hp in range(HPAIRS):
                heads = (2 * hp, 2 * hp + 1)
                bc_ps = ps_acc.tile([P, NQ], F32, tag="den", name=f"bc_{j}_{hp}")
                for i, h in enumerate(heads):
                    nc.tensor.matmul(
                        bc_ps[ds(i * dk, dk), :],
                        onesf[ds(32 * h, 1), :dk],
                        rec[ds(32 * h, 1), :],
                        start=True,
                        stop=True,
                        tile_position=(32 * h, i * dk),
                        skip_group_check=True,
                    )
                bc_sb = smalls.tile([P, NQ], F32, tag="bcs")
                nc.scalar.copy(bc_sb[:], bc_ps[:])
                nc.vector.tensor_tensor(
                    ctxT[:, hp, :],
                    ctx_ps[hp][:],
                    bc_sb[:],
                    op=mybir.AluOpType.mult,
                )

            if phases == 2:
                nc.sync.dma_start(dbg["ctxT"][:, :, ds(j * NQ, NQ)], ctxT[:])
                continue

            # out-proj for this q-chunk: 4 row-chunks of 128
            # last q-chunk: nothing overlaps phase 3, so cycle po through the
            # freed attention accumulator banks to pipeline the tail
            po_pools = (
                [(ps_gen, "pp"), (ps_acc, "ctx0"), (ps_acc, "ctx1"), (ps_acc, "den")]
                if j == JC - 1
                else [(ps_gen, "pp")]
            )
            po_i = 0
            for scl in range(NQ // P):
                ot = outp.tile([P, d], ODT, tag="ot", name=f"ot_{j}_{scl}")
                for nj in range(d // NQ):
                    pool_, tag_ = po_pools[po_i % len(po_pools)]
                    po_i += 1
                    po = pool_.tile([P, NQ], F32, tag=tag_, name=f"po_{j}_{scl}_{nj}")
                    for kc in range(GDC):
                        nc.tensor.matmul(
                            po[:],
                            ctxT[:, kc, ts(scl, P)],
                            w_o(wt, kc)[:, ds(nj * NQ, NQ)],
                            start=(kc == 0),
                            stop=(kc == GDC - 1),
                        )
                    nc.vector.tensor_copy(ot[:, ds(nj * NQ, NQ)], po[:])
                nc.sync.dma_start(out[ts(j * (NQ // P) + scl, P), :], ot[:])

    if sim:
        nc.compile()
    else:
        nc.finalize()
    return nc


_NC_CACHE = {}


def get_nc(**kw):
    key = tuple(sorted(kw.items()))
    if key not in _NC_CACHE:
        _NC_CACHE[key] = build_nc(**kw)
    return _NC_CACHE[key]


def _bf16(a):
    import ml_dtypes

    return np.ascontiguousarray(a.astype(ml_dtypes.bfloat16))


def shard_inputs(q, k, v, mask, Wq, Wk, Wv, Wo):
    q = np.asarray(q, dtype=np.float32)
    k = np.asarray(k, dtype=np.float32)
    v = np.asarray(v, dtype=np.float32)
    mask = np.asarray(mask, dtype=np.int32)
    Wq, Wk, Wv, Wo = (np.asarray(w, dtype=np.float32) for w in (Wq, Wk, Wv, Wo))

    # per-batch shared tensors
    xT3 = [_bf16(np.stack([q[b].T, k[b].T, v[b].T])) for b in range(B)]
    mTb = [_bf16(mask[b, 0].T.astype(np.float32)) for b in range(B)]

    def pack_w(rows):
        parts = []
        for W in (Wq, Wk, Wv):
            wT = W[rows, :].T  # [D, GD]
            parts.append(wT.reshape(D // P, P, GD).transpose(1, 0, 2).reshape(P, -1))
        woT = Wo[:, rows].T  # [GD, D]
        parts.append(woT.reshape(GD // P, P, D).transpose(1, 0, 2).reshape(P, -1))
        return _bf16(np.concatenate(parts, axis=1))

    in_maps = []
    for c in range(NCORES):
        b, g = divmod(c, NCORES // B)
        rows = slice(g * GD, (g + 1) * GD)
        in_maps.append({"xT3": xT3[b], "maskT": mTb[b], "wpk": pack_w(rows)})
    return in_maps


def kernel(q, k, v, mask, Wq, Wk, Wv, Wo):
    from concourse.bass_utils import run_bass_kernel_spmd

    nc = get_nc()
    in_maps = shard_inputs(q, k, v, mask, Wq, Wk, Wv, Wo)
    res = run_bass_kernel_spmd(nc, in_maps, list(range(NCORES))).results
    out = np.zeros((B, S, D), dtype=np.float32)
    for c in range(NCORES):
        out[c // (NCORES // B)] += np.asarray(res[c]["out"]).astype(np.float32)
    return out


if __name__ == "__main__":
    nc = build_nc()
    print("built ok")
